# revision 35
# baseline (speedup 1.0000x reference)
"""Multi-head self-attention (RoPE, causal) Trainium2 Bass kernel.

Problem: B=4, S=2048, D=1024, H=16 heads, d_k=64, f32 in/out.

Sharding: head-parallel across 8 NeuronCores. Core c owns heads {2c, 2c+1}
and all batches/tokens. QKV projections are column-parallel (each core
computes only its heads' features), attention is fully local per core, and
the output projection is column-parallel after per-(batch, seq-half)
AllGathers of the per-core attention outputs (each core computes 128 of
the 1024 output features).

Layouts (transposed activations, [feature, token]):
  - host pre-transposes x to xT [D, B*S]
  - Q/K projections produce de-interleaved features (x1 = even dims, x2 =
    odd dims, per head); batched SBUF->SBUF dup DMAs duplicate each 32-row
    group so each head's rope'd features are 64 contiguous rows, with
    QR = XA*A + XB*B against host-built phase-interleaved cos/sin tables;
    scores q.k are invariant under the shared q/k permutation
  - scores are computed transposed, S^T [k-partitions, q-free], so softmax
    exp output P^T feeds the P@V matmul with no transposes
  - V is projected transposed then PE-transposed to [token, feature] with a
    per-head ones column so each P@V matmul also emits the softmax
    denominator as output row 64
  - normalization per seq half: denominator rows gathered, one approximate
    reciprocal, selector-matmul broadcast to [128, QC], in-place multiply

Schedule (the perf-critical part; PE has a p-state ramp, so stalls cost
double — the whole program is one software-pipelined stream per batch):
  - stream b = attention(b) + interleaved projection chains of batch b+1
    (front-loaded into the first 70%) + w_o of batch b-1 + the deferred
    second-half normalize of batch b-1
  - attention runs over (q-chunk, head, k-block-pair) tiles with score
    matmuls issued two pairs ahead of the exp/mask/PV consumers, so the
    in-order PE queue never sits behind the scalar engine
  - exp is a single wide 1024-col Activation per untrimmed pair (3D AP
    flattened via rearrange); trimmed pairs share one trim offset
  - elementwise work is balanced: DVE does rope/masks/finalize copies,
    the scalar engine does exp + projection PSUM casts, Pool only fires
    collectives
  - AllGathers are per seq-half and launch mid-stream; w_o consumes them
    a full stream later, so collective latency and fabric traffic hide
    under compute
"""

import numpy as np
import ml_dtypes
from contextlib import ExitStack

import concourse.bass as bass
import concourse.bacc as bacc
import concourse.tile as tile
from concourse import mybir
from concourse.bass_utils import run_bass_kernel_spmd
from concourse.masks import make_upper_triangular, make_identity

F32 = mybir.dt.float32
BF16 = mybir.dt.bfloat16

B, S, D, H = 4, 2048, 1024, 16
NC = 8
DK = 64
HPC = H // NC  # heads per core
THETA = 100000.0
QC = 512       # q chunk (tokens per score-strip column block)
KB = 128       # k block (tokens per score-strip partition block)
NDB = D // 128  # number of 128-wide contraction blocks

ts = bass.ts
ds = bass.ds


def build_program(seq=S, mm_dt=BF16, n_cores=NC):
    """Build the SPMD Bass/Tile program (identical on every core)."""
    nc = bacc.Bacc("TRN2", target_bir_lowering=False, debug=False,
                   num_devices=n_cores)
    T = B * seq
    nqc = seq // QC
    nkb = seq // KB
    ntb = T // KB

    xT_h = nc.declare_dram_parameter("xT", [D, T], mm_dt, isOutput=False)
    wqk_h = nc.declare_dram_parameter("wqkT", [128, 2, NDB, 128], mm_dt,
                                      isOutput=False)
    wv_h = nc.declare_dram_parameter("wvT", [128, NDB, 128], mm_dt, isOutput=False)
    wo_h = nc.declare_dram_parameter("woT", [128, NDB, 128], mm_dt, isOutput=False)
    ra_h = nc.declare_dram_parameter("ropeA", [128, seq], mm_dt, isOutput=False)
    rb_h = nc.declare_dram_parameter("ropeB", [128, seq], mm_dt, isOutput=False)
    sel_h = nc.declare_dram_parameter("selT", [2 * HPC, nqc, 128],
                                      mm_dt, isOutput=False)
    selq_h = nc.declare_dram_parameter("selQ", [HPC, 128], mm_dt,
                                       isOutput=False)
    yT_h = nc.declare_dram_parameter("yT", [128, T], F32, isOutput=True)

    EXP = mybir.ActivationFunctionType.Exp
    scale = 1.0 / float(np.sqrt(DK))
    NKP = QC // KB  # k-blocks per q-chunk width

    with tile.TileContext(nc, num_cores=n_cores) as tc, ExitStack() as ctx:
        consts = ctx.enter_context(tc.tile_pool(name="consts", bufs=1))

        wqks = consts.tile([128, 2, NDB, 128], mm_dt)
        nc.sync.dma_start(wqks[:], wqk_h[:, :, :, :])
        wvs = consts.tile([128, NDB, 128], mm_dt)
        nc.sync.dma_start(wvs[:], wv_h[:, :, :])
        ra_t = consts.tile([128, seq], mm_dt)
        rb_t = consts.tile([128, seq], mm_dt)
        wos = consts.tile([128, NDB, 128], mm_dt)
        sels = consts.tile([2 * HPC, nqc, 128], mm_dt)
        selq = consts.tile([HPC, 128], mm_dt)

        def load_late_consts():
            nc.sync.dma_start(ra_t[:], ra_h[:, :])
            nc.sync.dma_start(rb_t[:], rb_h[:, :])
            nc.sync.dma_start(wos[:], wo_h[:, :, :])
            nc.sync.dma_start(sels[:], sel_h[:, :, :])
            nc.sync.dma_start(selq[:], selq_h[:, :])

        triu = consts.tile([128, 128], mm_dt)
        make_upper_triangular(nc, triu[:], val=1.0, diag=True)
        ident = consts.tile([128, 128], mm_dt)
        make_identity(nc, ident[:])

        NTB = seq // KB   # 128-token V blocks per batch
        SH = seq // 2     # half-batch tokens

        xpool = ctx.enter_context(tc.tile_pool(name="xcp", bufs=2))
        stg = ctx.enter_context(tc.tile_pool(name="stage", bufs=1))
        dpup = ctx.enter_context(tc.tile_pool(name="dupt", bufs=1))
        rpool = ctx.enter_context(tc.tile_pool(name="ropet", bufs=4))
        vtp = ctx.enter_context(tc.tile_pool(name="vtcp", bufs=2))
        qrp = ctx.enter_context(tc.tile_pool(name="qrp", bufs=2))
        krp = ctx.enter_context(tc.tile_pool(name="krp", bufs=2))
        vap = ctx.enter_context(tc.tile_pool(name="vap", bufs=2))
        attp = ctx.enter_context(tc.tile_pool(name="attp", bufs=2))
        sps = ctx.enter_context(tc.tile_pool(name="sps", bufs=2, space="PSUM"))
        pvps = ctx.enter_context(tc.tile_pool(name="pvps", bufs=2,
                                              space="PSUM"))
        aux = ctx.enter_context(tc.tile_pool(name="auxps", bufs=2,
                                             space="PSUM"))
        ptp = ctx.enter_context(tc.tile_pool(name="ptp", bufs=4))
        nrm = ctx.enter_context(tc.tile_pool(name="nrm", bufs=2))
        dpool = ctx.enter_context(tc.tile_pool(name="dram", bufs=1,
                                               space="DRAM"))
        agp = ctx.enter_context(tc.tile_pool(name="agp", bufs=3))
        ysb = ctx.enter_context(tc.tile_pool(name="ysb", bufs=2))
        if True:
            # per-(batch, half) collective buffers
            attbs = [[dpool.tile([128, SH], mm_dt, name=f"attb{_b}h{_h}")
                      for _h in range(2)] for _b in range(B)]
            agbs = [[dpool.tile([128 * n_cores, SH], mm_dt,
                                addr_space="Shared", name=f"agb{_b}h{_h}")
                     for _h in range(2)] for _b in range(B)]

            xr = xT_h[:, :].rearrange("(i p) t -> p i t", p=128)

            # per-batch rotating tiles (dicts keyed by batch)
            QRs, KRs, VAs, ATTs = {}, {}, {}, {}

            # ---- w_o helpers (chunk cc of batch bb) ----------------------
            def wo_load(bb, cc):
                agr = agbs[bb][cc // 2][:, :].rearrange(
                    "(i p) t -> p i t", p=128)
                rt = agp.tile([128, NDB, QC], mm_dt, tag="rt", name="rt")
                nc.sync.dma_start(rt[:], agr[:, :, ts(cc % 2, QC)])
                return rt

            def wo_mm(bb, cc, rt):
                yp = aux.tile([128, QC], F32, tag="aux", name="yp")
                for i in range(NDB):
                    nc.tensor.matmul(yp[:], wos[:, i, :], rt[:, i, :],
                                     start=(i == 0), stop=(i == NDB - 1))
                ysb_t = ysb.tile([128, QC], F32, tag="ys", name="ysb_t")
                nc.vector.tensor_copy(ysb_t[:], yp[:])
                nc.sync.dma_start(yT_h[:, ds(bb * seq + cc * QC, QC)],
                                  ysb_t[:])

            # ---- attention pair helpers (2 k-blocks per PSUM tile) ------
            def score_pair(b, hl, qi, s):
                hr = ds(64 * hl, 64)
                sp = sps.tile([128, 2, QC], F32, tag="sp", name="sp")
                for j in range(2):
                    kb = 2 * s + j
                    ksl = ds(kb * KB, KB)
                    dj = kb - qi * NKP
                    qv = 128 * dj if dj > 0 else 0
                    qslv = ds(qi * QC + qv, QC - qv)
                    nc.tensor.matmul(sp[:, j, qv:QC], KRs[b][hr, ksl],
                                     QRs[b][hr, qslv], start=True, stop=True)
                return sp

            def expmask_pair(b, hl, qi, s, sp):
                pt = ptp.tile([128, 2, QC], mm_dt, tag="pt", name="pt")
                djs = [2 * s + j - qi * NKP for j in range(2)]
                qvm = max(0, min(djs)) * 128
                if qvm == 0:  # one wide 2D activation over the whole pair
                    nc.scalar.activation(
                        pt[:, :, :].rearrange("p a c -> p (a c)"),
                        sp[:, :, :].rearrange("p a c -> p (a c)"),
                        EXP, scale=scale)
                else:
                    nc.scalar.activation(pt[:, :, qvm:QC], sp[:, :, qvm:QC],
                                         EXP, scale=scale)
                for j in range(2):
                    if djs[j] >= 0:  # diagonal block: causal mask
                        dsl = ds(128 * djs[j], 128)
                        nc.vector.tensor_mul(pt[:, j, dsl], pt[:, j, dsl],
                                             triu[:])
                return pt

            def pv_pair(b, hl, qi, s, pt, pv, nk):
                for j in range(2):
                    kb = 2 * s + j
                    dj = kb - qi * NKP
                    qv = 128 * dj if dj > 0 else 0
                    nc.tensor.matmul(pv[:, qv:QC],
                                     VAs[b][:, kb, ds(65 * hl, 65)],
                                     pt[:, j, qv:QC],
                                     start=(kb == 0), stop=(kb == nk - 1))

            # ---- projection task list for batch bn ----------------------
            def make_proj_tasks(bn):
                """Return a list of callables; run in order (interleaved)."""
                st = {"xcs": {}, "xs": None, "xd": None}

                def alloc():
                    QRs[bn] = qrp.tile([128, seq], mm_dt, tag="qr",
                                       name=f"qr{bn}")
                    KRs[bn] = krp.tile([128, seq], mm_dt, tag="kr",
                                       name=f"kr{bn}")
                    VAs[bn] = vap.tile([128, NTB, 130], mm_dt, tag="va",
                                       name=f"va{bn}")
                    ATTs[bn] = attp.tile([128, seq], mm_dt, tag="att",
                                         name=f"att{bn}")
                    nc.vector.memset(VAs[bn][:, :, 64], 1.0)
                    nc.vector.memset(VAs[bn][:, :, 129], 1.0)
                    st["xs"] = stg.tile([128, 2, seq], mm_dt, tag="xs",
                                        name="xs")
                    st["xd"] = dpup.tile([128, 4, seq], mm_dt, tag="xd",
                                         name="xd")
                    xc = xpool.tile([128, NDB, QC], mm_dt, tag="xc",
                                    name="xc")
                    nc.sync.dma_start(xc[:], xr[:, :, ts(bn * nqc, QC)])
                    st["xcs"][0] = xc

                def chain_v(ci, half):
                    if half == 0:
                        if ci + 1 < nqc:  # prefetch next chunk
                            xc = xpool.tile([128, NDB, QC], mm_dt, tag="xc",
                                            name="xc")
                            nc.sync.dma_start(
                                xc[:], xr[:, :, ts(bn * nqc + ci + 1, QC)])
                            st["xcs"][ci + 1] = xc
                        st["psv"] = aux.tile([128, QC], F32, tag="aux",
                                             name="psv")
                    psv = st["psv"]
                    for i in range(4 * half, 4 * half + 4):
                        nc.tensor.matmul(psv[:], wvs[:, i, :],
                                         st["xcs"][ci][:, i, :],
                                         start=(i == 0), stop=(i == NDB - 1))
                    if half == 1:
                        vtc = vtp.tile([128, QC], mm_dt, tag="vtc",
                                       name="vtc")
                        nc.scalar.copy(vtc[:], psv[:])
                        st[f"vtc{ci}"] = vtc

                def chain_x(ci, g, half):
                    if half == 0:
                        st["ps"] = aux.tile([128, QC], F32, tag="aux",
                                            name="ps")
                    ps = st["ps"]
                    for i in range(4 * half, 4 * half + 4):
                        nc.tensor.matmul(ps[:], wqks[:, g, i, :],
                                         st["xcs"][ci][:, i, :],
                                         start=(i == 0), stop=(i == NDB - 1))
                    if half == 1:
                        nc.vector.tensor_copy(st["xs"][:, g, ds(ci * QC, QC)],
                                              ps[:])

                def transpose_v(ci):
                    vtc = st[f"vtc{ci}"]
                    for sb in range(QC // 128):
                        tb = ci * (QC // 128) + sb
                        tp = aux.tile([128, 128], mm_dt, tag="aux", name="tp")
                        nc.tensor.transpose(tp[:], vtc[:, ts(sb, 128)],
                                            ident[:])
                        nc.vector.tensor_copy(VAs[bn][:, tb, 0:64],
                                              tp[:, 0:64])
                        nc.vector.tensor_copy(VAs[bn][:, tb, 65:129],
                                              tp[:, 64:128])

                def dup_half(hb):
                    hsl = ds(hb * SH, SH)
                    for t_i, (g, base) in enumerate(
                            ((0, 0), (1, 0), (0, 64), (1, 64))):
                        for blk in range(2):
                            for du in range(2):
                                nc.sync.dma_start(
                                    st["xd"][ds(64 * blk + 32 * du, 32),
                                             t_i, hsl],
                                    st["xs"][ds(base + 32 * blk, 32), g, hsl])

                def rope_q(cq):
                    qsl = ds(cq * QC, QC)
                    for xi, OUT in ((0, QRs[bn]), (2, KRs[bn])):
                        t1 = rpool.tile([128, QC], F32, tag=f"t1{xi}",
                                        name="t1")
                        t2 = rpool.tile([128, QC], F32, tag=f"t2{xi}",
                                        name="t2")
                        nc.gpsimd.tensor_mul(t1[:], st["xd"][:, xi, qsl],
                                             ra_t[:, qsl])
                        nc.vector.tensor_mul(t2[:], st["xd"][:, xi + 1, qsl],
                                             rb_t[:, qsl])
                        nc.vector.tensor_add(OUT[:, qsl], t1[:], t2[:])

                tasks = [alloc]
                for hb in range(2):
                    for ci in (2 * hb, 2 * hb + 1):
                        for half in range(2):
                            tasks.append(
                                lambda ci=ci, half=half: chain_v(ci, half))
                        for g in range(2):
                            for half in range(2):
                                tasks.append(lambda ci=ci, g=g, half=half:
                                             chain_x(ci, g, half))
                        tasks.append(lambda ci=ci: transpose_v(ci))
                    tasks.append(lambda hb=hb: dup_half(hb))
                    tasks.append(lambda hb=hb: rope_q(2 * hb))
                    tasks.append(lambda hb=hb: rope_q(2 * hb + 1))
                return tasks

            # ---- normalize half h of batch b ----------------------------
            def norm_recip(b, h):
                dbt = nrm.tile([2 * HPC, QC], F32, tag=f"dbt{h}",
                               name=f"dbt{h}")
                return dbt

            def norm_rest(b, h, dbt):
                rbf = nrm.tile([2 * HPC, QC], F32, tag=f"rbf{h}",
                               name=f"rbf{h}")
                rbt = nrm.tile([2 * HPC, QC], mm_dt,
                               tag=f"rbt{h}", name=f"rbt{h}")
                with nc.allow_low_precision(reason="softmax denom bcast"):
                    nc.vector.reciprocal_approx_fast(rbf[:], dbt[:])
                    nc.vector.tensor_copy(rbt[:], rbf[:])
                for qi in (2 * h, 2 * h + 1):
                    qsl = ds(qi * QC, QC)
                    bp = aux.tile([128, QC], F32, tag="aux", name="bp")
                    nc.tensor.matmul(bp[:], sels[:, qi, :], rbt[:],
                                     start=True, stop=True)
                    nc.vector.tensor_mul(ATTs[b][:, qsl], ATTs[b][:, qsl],
                                         bp[:])
                nc.sync.dma_start(attbs[b][h][:, :],
                                  ATTs[b][:, ds(h * SH, SH)])
                nc.gpsimd.collective_compute(
                    "AllGather", mybir.AluOpType.bypass,
                    replica_groups=[list(range(n_cores))],
                    ins=[attbs[b][h][:, :].opt()],
                    outs=[agbs[b][h][:, :].opt()])

            # ---- one merged stream per batch ----------------------------
            # stream b = attention(b) + interleaved proj(b+1) + w_o(b-1)
            #            + deferred normalize
            DEPTH = 2
            # batch 0: emit only the first seq-half's projection up front
            # (attention qi<2 needs just rope(0,1)); the second half rides
            # stream 0's interleave ahead of batch 1's projection
            proj0 = make_proj_tasks(0)
            proj0[0]()          # alloc + first xc DMA
            load_late_consts()  # rope tables etc. queue after xc(0)
            nh0 = 1 + (len(proj0) - 1) // 2
            for t in proj0[1:nh0]:
                t()
            carry_proj = proj0[nh0:]

            carry_norm = None  # (b, h, dbt) deferred into next stream
            rt3 = {}
            for b in range(B):
                blocks = []
                for qi in range(nqc):
                    npair = (qi + 1) * NKP // 2
                    for hl in range(HPC):
                        for s in range(npair):
                            blocks.append((hl, qi, s, npair))
                ptasks = ((carry_proj if b == 0 else []) +
                          (make_proj_tasks(b + 1) if b + 1 < B else []))
                # interleave positions: proj task p after block idx
                # p*len(blocks)//len(ptasks)
                nxt_p = 0
                last = b == B - 1
                dbtA = norm_recip(b, 0)
                dbtB = norm_recip(b, 1)
                dbts = (dbtA, dbtB)
                pend = []  # (due_idx, fn)
                spq = [score_pair(b, *blocks[i][:3]) for i in range(DEPTH)]
                pv = None
                rt_pend = None
                for idx, (hl, qi, s, npair) in enumerate(blocks):
                    sp_cur = spq.pop(0)
                    if idx + DEPTH < len(blocks):
                        spq.append(score_pair(b, *blocks[idx + DEPTH][:3]))
                    pt = expmask_pair(b, hl, qi, s, sp_cur)
                    if s == 0:
                        pv = pvps.tile([65, QC], F32, tag="pv", name="pv")
                    pv_pair(b, hl, qi, s, pt, pv, npair * 2)
                    if s == npair - 1:  # finalize (qi, hl)
                        qsl = ds(qi * QC, QC)
                        nc.vector.tensor_copy(ATTs[b][ds(64 * hl, 64), qsl],
                                              pv[ds(0, 64), :])
                        deng = nrm.tile([1, QC], F32, tag="deng",
                                        name="deng")
                        nc.vector.tensor_copy(deng[:], pv[ds(64, 1), :])
                        nc.sync.dma_start(
                            dbts[qi // 2][ds((qi % 2) * HPC + hl, 1), :],
                            deng[:])
                        if qi == 1 and hl == HPC - 1:
                            # half A fully done: normalize soon after
                            pend.append((idx + 2,
                                         lambda: norm_rest(b, 0, dbtA)))
                        # w_o of previous batch rides in this stream;
                        # for the last batch its own w_o starts in-stream
                        # as the quarter AllGathers land
                        if b > 0:
                            if hl == 0 and qi == 0:
                                rt_pend = wo_load(b - 1, 0)
                            elif hl == HPC - 1:
                                rt_next = (wo_load(b - 1, qi + 1)
                                           if qi + 1 < nqc else None)
                                wo_mm(b - 1, qi, rt_pend)
                                rt_pend = rt_next
                        # last batch: its own first-half w_o rides the
                        # end of its stream (AG h0 launched after qi=1)
                        if last and qi == 3:
                            if hl == 0:
                                rt3[0] = wo_load(b, 0)
                            else:
                                rt3[1] = wo_load(b, 1)
                                wo_mm(b, 0, rt3[0])
                    # deferred normalize from this or the previous stream
                    while pend and pend[0][0] <= idx:
                        pend.pop(0)[1]()
                    if carry_norm is not None and idx >= 1:
                        cb, ch, cdbt = carry_norm
                        carry_norm = None
                        norm_rest(cb, ch, cdbt)
                    # interleaved projection of the next batch
                    while (ptasks and
                           nxt_p < len(ptasks) and
                           idx >= nxt_p * (6 * len(blocks) // 10)
                           // len(ptasks)):
                        ptasks[nxt_p]()
                        nxt_p += 1
                while nxt_p < len(ptasks):
                    ptasks[nxt_p]()
                    nxt_p += 1
                for due, fn in pend:
                    fn()
                if b + 1 < B:
                    carry_norm = (b, 1, dbtB)
                else:
                    norm_rest(b, 1, dbtB)

            # tail: second-half w_o of the last batch (loads ahead)
            wo_mm(B - 1, 1, rt3[1])
            rt3[2] = wo_load(B - 1, 2)
            rt3[3] = wo_load(B - 1, 3)
            wo_mm(B - 1, 2, rt3[2])
            wo_mm(B - 1, 3, rt3[3])

    nc.compile()
    return nc


def prep_inputs(inputs, seq=S, mm_dt=BF16, n_cores=NC):
    """Host-side sharding: build the per-core input maps."""
    mm_np = ml_dtypes.bfloat16 if mm_dt == BF16 else np.float32
    x = np.asarray(inputs["in_features"], dtype=np.float32)
    pos = np.asarray(inputs["token_positions"]).astype(np.float32)
    wq = np.asarray(inputs["w_q"], dtype=np.float32)
    wk = np.asarray(inputs["w_k"], dtype=np.float32)
    wv = np.asarray(inputs["w_v"], dtype=np.float32)
    wo = np.asarray(inputs["w_o"], dtype=np.float32)

    T = B * seq
    xT = np.ascontiguousarray(x.reshape(T, D).T).astype(mm_np)

    # rope tables (f32, matching reference numerics):
    #   QR = XA*A + XB*B ; A rows per 32-block: [cos, sin]*2 ;
    #   B rows: [-sin, cos]*2   (r2(t) = r1(t - pi/2))
    inv = np.float32(THETA) ** (-np.arange(0, DK, 2, dtype=np.float32)
                                / np.float32(DK))
    ang = pos[:, None].astype(np.float32) * inv[None, :].astype(np.float32)
    cosT = np.cos(ang.astype(np.float32)).T  # [32, seq]
    sinT = np.sin(ang.astype(np.float32)).T
    ropeA = np.ascontiguousarray(
        np.concatenate([cosT, sinT, cosT, sinT], axis=0)).astype(mm_np)
    ropeB = np.ascontiguousarray(
        np.concatenate([-sinT, cosT, -sinT, cosT], axis=0)).astype(mm_np)

    # normalize broadcast selectors (per seq half): out row m of chunk qi
    # takes denominator group g = (qi % 2) * HPC + (m // 64)
    nqc = seq // QC
    selT = np.zeros((2 * HPC, nqc, 128), dtype=mm_np)
    for qi in range(nqc):
        for m in range(128):
            selT[(qi % 2) * HPC + (m // 64), qi, m] = 1.0
    selQ = np.zeros((HPC, 128), dtype=mm_np)
    for m in range(128):
        selQ[m // 64, m] = 1.0

    ev = np.arange(0, DK, 2)
    od = ev + 1

    def lhsT_stack(W):
        # W [128 out-features, D] -> [128, NDB, 128]; [:, i, :] = W[:, 128i:+128].T
        Wt = np.ascontiguousarray(W.T).astype(mm_np)  # [D, 128]
        return np.ascontiguousarray(
            Wt.reshape(NDB, 128, 128).transpose(1, 0, 2))

    in_maps = []
    for c in range(n_cores):
        h0, h1 = HPC * c, HPC * c + 1
        W1 = np.concatenate([wq[DK * h0 + ev], wq[DK * h1 + ev],
                             wk[DK * h0 + ev], wk[DK * h1 + ev]], axis=0)
        W2 = np.concatenate([wq[DK * h0 + od], wq[DK * h1 + od],
                             wk[DK * h0 + od], wk[DK * h1 + od]], axis=0)
        wqkT = np.ascontiguousarray(np.stack(
            [lhsT_stack(Wg) for Wg in (W1, W2)], axis=1))
        WV = wv[128 * c: 128 * (c + 1)]
        WO = wo[128 * c: 128 * (c + 1)]
        in_maps.append({
            "xT": xT,
            "wqkT": wqkT,
            "wvT": lhsT_stack(WV),
            "woT": lhsT_stack(WO),
            "ropeA": ropeA,
            "ropeB": ropeB,
            "selT": selT,
            "selQ": selQ,
        })
    return in_maps


def assemble_output(results, seq=S, n_cores=NC):
    yT = np.concatenate([np.asarray(r["yT"], dtype=np.float32)
                         for r in results], axis=0)  # [1024, T]
    return np.ascontiguousarray(yT.T).reshape(B, seq, D).astype(np.float32)


_PROGRAM_CACHE = {}


def kernel(**inputs) -> np.ndarray:
    key = ("full", S, "bf16")
    if key not in _PROGRAM_CACHE:
        _PROGRAM_CACHE[key] = build_program(seq=S, mm_dt=BF16, n_cores=NC)
    nc = _PROGRAM_CACHE[key]
    in_maps = prep_inputs(inputs, seq=S, mm_dt=BF16, n_cores=NC)
    res = run_bass_kernel_spmd(nc, in_maps, list(range(NC)))
    return assemble_output(res.results)


# revision 36
# speedup vs baseline: 1.0108x; 1.0108x over previous
"""Multi-head self-attention (RoPE, causal) Trainium2 Bass kernel.

Problem: B=4, S=2048, D=1024, H=16 heads, d_k=64, f32 in/out.

Sharding: head-parallel across 8 NeuronCores. Core c owns heads {2c, 2c+1}
and all batches/tokens. QKV projections are column-parallel (each core
computes only its heads' features), attention is fully local per core, and
the output projection is column-parallel after per-(batch, seq-half)
AllGathers of the per-core attention outputs (each core computes 128 of
the 1024 output features).

Layouts (transposed activations, [feature, token]):
  - host pre-transposes x to xT [D, B*S]
  - Q/K projections produce de-interleaved features (x1 = even dims, x2 =
    odd dims, per head); batched SBUF->SBUF dup DMAs duplicate each 32-row
    group so each head's rope'd features are 64 contiguous rows, with
    QR = XA*A + XB*B against host-built phase-interleaved cos/sin tables;
    scores q.k are invariant under the shared q/k permutation
  - scores are computed transposed, S^T [k-partitions, q-free], so softmax
    exp output P^T feeds the P@V matmul with no transposes
  - V is projected transposed then PE-transposed to [token, feature] with a
    per-head ones column so each P@V matmul also emits the softmax
    denominator as output row 64
  - normalization per seq half: denominator rows gathered, one approximate
    reciprocal, selector-matmul broadcast to [128, QC], in-place multiply

Schedule (the perf-critical part; PE has a p-state ramp, so stalls cost
double — the whole program is one software-pipelined stream per batch):
  - stream b = attention(b) + interleaved projection chains of batch b+1
    (front-loaded into the first 70%) + w_o of batch b-1 + the deferred
    second-half normalize of batch b-1
  - attention runs over (q-chunk, head, k-block-pair) tiles with score
    matmuls issued two pairs ahead of the exp/mask/PV consumers, so the
    in-order PE queue never sits behind the scalar engine
  - exp is a single wide 1024-col Activation per untrimmed pair (3D AP
    flattened via rearrange); trimmed pairs share one trim offset
  - elementwise work is balanced: DVE does rope/masks/finalize copies,
    the scalar engine does exp + projection PSUM casts, Pool only fires
    collectives
  - AllGathers are per seq-half and launch mid-stream; w_o consumes them
    a full stream later, so collective latency and fabric traffic hide
    under compute
"""

import numpy as np
import ml_dtypes
from contextlib import ExitStack

import concourse.bass as bass
import concourse.bacc as bacc
import concourse.tile as tile
from concourse import mybir
from concourse.bass_utils import run_bass_kernel_spmd
from concourse.masks import make_upper_triangular, make_identity

F32 = mybir.dt.float32
BF16 = mybir.dt.bfloat16

B, S, D, H = 4, 2048, 1024, 16
NC = 8
DK = 64
HPC = H // NC  # heads per core
THETA = 100000.0
QC = 512       # q chunk (tokens per score-strip column block)
KB = 128       # k block (tokens per score-strip partition block)
NDB = D // 128  # number of 128-wide contraction blocks

ts = bass.ts
ds = bass.ds


def build_program(seq=S, mm_dt=BF16, n_cores=NC):
    """Build the SPMD Bass/Tile program (identical on every core)."""
    nc = bacc.Bacc("TRN2", target_bir_lowering=False, debug=False,
                   num_devices=n_cores)
    T = B * seq
    nqc = seq // QC
    nkb = seq // KB
    ntb = T // KB

    xT_h = nc.declare_dram_parameter("xT", [D, T], mm_dt, isOutput=False)
    wqk_h = nc.declare_dram_parameter("wqkT", [128, 2, NDB, 128], mm_dt,
                                      isOutput=False)
    wv_h = nc.declare_dram_parameter("wvT", [128, NDB, 128], mm_dt, isOutput=False)
    wo_h = nc.declare_dram_parameter("woT", [128, NDB, 128], mm_dt, isOutput=False)
    ra_h = nc.declare_dram_parameter("ropeA", [128, seq], mm_dt, isOutput=False)
    rb_h = nc.declare_dram_parameter("ropeB", [128, seq], mm_dt, isOutput=False)
    sel_h = nc.declare_dram_parameter("selT", [2 * HPC, nqc, 128],
                                      mm_dt, isOutput=False)
    selq_h = nc.declare_dram_parameter("selQ", [HPC, 128], mm_dt,
                                       isOutput=False)
    yT_h = nc.declare_dram_parameter("yT", [128, T], F32, isOutput=True)

    EXP = mybir.ActivationFunctionType.Exp
    scale = 1.0 / float(np.sqrt(DK))
    NKP = QC // KB  # k-blocks per q-chunk width

    with tile.TileContext(nc, num_cores=n_cores) as tc, ExitStack() as ctx:
        consts = ctx.enter_context(tc.tile_pool(name="consts", bufs=1))

        wqks = consts.tile([128, 2, NDB, 128], mm_dt)
        nc.sync.dma_start(wqks[:], wqk_h[:, :, :, :])
        wvs = consts.tile([128, NDB, 128], mm_dt)
        nc.sync.dma_start(wvs[:], wv_h[:, :, :])
        ra_t = consts.tile([128, seq], mm_dt)
        rb_t = consts.tile([128, seq], mm_dt)
        wos = consts.tile([128, NDB, 128], mm_dt)
        sels = consts.tile([2 * HPC, nqc, 128], mm_dt)
        selq = consts.tile([HPC, 128], mm_dt)

        def load_late_consts():
            nc.sync.dma_start(ra_t[:], ra_h[:, :])
            nc.sync.dma_start(rb_t[:], rb_h[:, :])
            nc.sync.dma_start(wos[:], wo_h[:, :, :])
            nc.sync.dma_start(sels[:], sel_h[:, :, :])
            nc.sync.dma_start(selq[:], selq_h[:, :])

        triu = consts.tile([128, 128], mm_dt)
        make_upper_triangular(nc, triu[:], val=1.0, diag=True)
        ident = consts.tile([128, 128], mm_dt)
        make_identity(nc, ident[:])

        NTB = seq // KB   # 128-token V blocks per batch
        SH = seq // 2     # half-batch tokens

        xpool = ctx.enter_context(tc.tile_pool(name="xcp", bufs=2))
        stg = ctx.enter_context(tc.tile_pool(name="stage", bufs=1))
        dpup = ctx.enter_context(tc.tile_pool(name="dupt", bufs=1))
        rpool = ctx.enter_context(tc.tile_pool(name="ropet", bufs=4))
        vtp = ctx.enter_context(tc.tile_pool(name="vtcp", bufs=2))
        qrp = ctx.enter_context(tc.tile_pool(name="qrp", bufs=2))
        krp = ctx.enter_context(tc.tile_pool(name="krp", bufs=2))
        vap = ctx.enter_context(tc.tile_pool(name="vap", bufs=2))
        attp = ctx.enter_context(tc.tile_pool(name="attp", bufs=2))
        sps = ctx.enter_context(tc.tile_pool(name="sps", bufs=2, space="PSUM"))
        pvps = ctx.enter_context(tc.tile_pool(name="pvps", bufs=2,
                                              space="PSUM"))
        aux = ctx.enter_context(tc.tile_pool(name="auxps", bufs=2,
                                             space="PSUM"))
        ptp = ctx.enter_context(tc.tile_pool(name="ptp", bufs=4))
        nrm = ctx.enter_context(tc.tile_pool(name="nrm", bufs=2))
        dpool = ctx.enter_context(tc.tile_pool(name="dram", bufs=1,
                                               space="DRAM"))
        agp = ctx.enter_context(tc.tile_pool(name="agp", bufs=3))
        ysb = ctx.enter_context(tc.tile_pool(name="ysb", bufs=2))
        if True:
            # per-(batch, half) collective buffers
            attbs = [[dpool.tile([128, SH], mm_dt, name=f"attb{_b}h{_h}")
                      for _h in range(2)] for _b in range(B)]
            agbs = [[dpool.tile([128 * n_cores, SH], mm_dt,
                                addr_space="Shared", name=f"agb{_b}h{_h}")
                     for _h in range(2)] for _b in range(B)]

            attbq = {_q: dpool.tile([128, QC], mm_dt, name=f"attbq{_q}")
                     for _q in (2, 3)}
            agbq = {_q: dpool.tile([128 * n_cores, QC], mm_dt,
                                   addr_space="Shared", name=f"agbq{_q}")
                    for _q in (2, 3)}
            xr = xT_h[:, :].rearrange("(i p) t -> p i t", p=128)

            # per-batch rotating tiles (dicts keyed by batch)
            QRs, KRs, VAs, ATTs = {}, {}, {}, {}

            # ---- w_o helpers (chunk cc of batch bb) ----------------------
            def wo_load(bb, cc):
                if bb == B - 1 and cc >= 2:
                    agr = agbq[cc][:, :].rearrange("(i p) t -> p i t", p=128)
                    csl = ts(0, QC)
                else:
                    agr = agbs[bb][cc // 2][:, :].rearrange(
                        "(i p) t -> p i t", p=128)
                    csl = ts(cc % 2, QC)
                rt = agp.tile([128, NDB, QC], mm_dt, tag="rt", name="rt")
                nc.sync.dma_start(rt[:], agr[:, :, csl])
                return rt

            def wo_mm(bb, cc, rt):
                yp = aux.tile([128, QC], F32, tag="aux", name="yp")
                for i in range(NDB):
                    nc.tensor.matmul(yp[:], wos[:, i, :], rt[:, i, :],
                                     start=(i == 0), stop=(i == NDB - 1))
                ysb_t = ysb.tile([128, QC], F32, tag="ys", name="ysb_t")
                nc.vector.tensor_copy(ysb_t[:], yp[:])
                nc.sync.dma_start(yT_h[:, ds(bb * seq + cc * QC, QC)],
                                  ysb_t[:])

            # ---- attention pair helpers (2 k-blocks per PSUM tile) ------
            def score_pair(b, hl, qi, s):
                hr = ds(64 * hl, 64)
                sp = sps.tile([128, 2, QC], F32, tag="sp", name="sp")
                for j in range(2):
                    kb = 2 * s + j
                    ksl = ds(kb * KB, KB)
                    dj = kb - qi * NKP
                    qv = 128 * dj if dj > 0 else 0
                    qslv = ds(qi * QC + qv, QC - qv)
                    nc.tensor.matmul(sp[:, j, qv:QC], KRs[b][hr, ksl],
                                     QRs[b][hr, qslv], start=True, stop=True)
                return sp

            def expmask_pair(b, hl, qi, s, sp):
                pt = ptp.tile([128, 2, QC], mm_dt, tag="pt", name="pt")
                djs = [2 * s + j - qi * NKP for j in range(2)]
                qvm = max(0, min(djs)) * 128
                if qvm == 0:  # one wide 2D activation over the whole pair
                    nc.scalar.activation(
                        pt[:, :, :].rearrange("p a c -> p (a c)"),
                        sp[:, :, :].rearrange("p a c -> p (a c)"),
                        EXP, scale=scale)
                else:
                    nc.scalar.activation(pt[:, :, qvm:QC], sp[:, :, qvm:QC],
                                         EXP, scale=scale)
                for j in range(2):
                    if djs[j] >= 0:  # diagonal block: causal mask
                        dsl = ds(128 * djs[j], 128)
                        nc.vector.tensor_mul(pt[:, j, dsl], pt[:, j, dsl],
                                             triu[:])
                return pt

            def pv_pair(b, hl, qi, s, pt, pv, nk):
                for j in range(2):
                    kb = 2 * s + j
                    dj = kb - qi * NKP
                    qv = 128 * dj if dj > 0 else 0
                    nc.tensor.matmul(pv[:, qv:QC],
                                     VAs[b][:, kb, ds(65 * hl, 65)],
                                     pt[:, j, qv:QC],
                                     start=(kb == 0), stop=(kb == nk - 1))

            # ---- projection task list for batch bn ----------------------
            def make_proj_tasks(bn):
                """Return a list of callables; run in order (interleaved)."""
                st = {"xcs": {}, "xs": None, "xd": None}

                def alloc():
                    QRs[bn] = qrp.tile([128, seq], mm_dt, tag="qr",
                                       name=f"qr{bn}")
                    KRs[bn] = krp.tile([128, seq], mm_dt, tag="kr",
                                       name=f"kr{bn}")
                    VAs[bn] = vap.tile([128, NTB, 130], mm_dt, tag="va",
                                       name=f"va{bn}")
                    ATTs[bn] = attp.tile([128, seq], mm_dt, tag="att",
                                         name=f"att{bn}")
                    nc.vector.memset(VAs[bn][:, :, 64], 1.0)
                    nc.vector.memset(VAs[bn][:, :, 129], 1.0)
                    st["xs"] = stg.tile([128, 2, seq], mm_dt, tag="xs",
                                        name="xs")
                    st["xd"] = dpup.tile([128, 4, seq], mm_dt, tag="xd",
                                         name="xd")
                    xc = xpool.tile([128, NDB, QC], mm_dt, tag="xc",
                                    name="xc")
                    nc.sync.dma_start(xc[:], xr[:, :, ts(bn * nqc, QC)])
                    st["xcs"][0] = xc

                def chain_v(ci, half):
                    if half == 0:
                        if ci + 1 < nqc:  # prefetch next chunk
                            xc = xpool.tile([128, NDB, QC], mm_dt, tag="xc",
                                            name="xc")
                            nc.sync.dma_start(
                                xc[:], xr[:, :, ts(bn * nqc + ci + 1, QC)])
                            st["xcs"][ci + 1] = xc
                        st["psv"] = aux.tile([128, QC], F32, tag="aux",
                                             name="psv")
                    psv = st["psv"]
                    for i in range(4 * half, 4 * half + 4):
                        nc.tensor.matmul(psv[:], wvs[:, i, :],
                                         st["xcs"][ci][:, i, :],
                                         start=(i == 0), stop=(i == NDB - 1))
                    if half == 1:
                        vtc = vtp.tile([128, QC], mm_dt, tag="vtc",
                                       name="vtc")
                        nc.scalar.copy(vtc[:], psv[:])
                        st[f"vtc{ci}"] = vtc

                def chain_x(ci, g, half):
                    if half == 0:
                        st["ps"] = aux.tile([128, QC], F32, tag="aux",
                                            name="ps")
                    ps = st["ps"]
                    for i in range(4 * half, 4 * half + 4):
                        nc.tensor.matmul(ps[:], wqks[:, g, i, :],
                                         st["xcs"][ci][:, i, :],
                                         start=(i == 0), stop=(i == NDB - 1))
                    if half == 1:
                        nc.vector.tensor_copy(st["xs"][:, g, ds(ci * QC, QC)],
                                              ps[:])

                def transpose_v(ci):
                    vtc = st[f"vtc{ci}"]
                    for sb in range(QC // 128):
                        tb = ci * (QC // 128) + sb
                        tp = aux.tile([128, 128], mm_dt, tag="aux", name="tp")
                        nc.tensor.transpose(tp[:], vtc[:, ts(sb, 128)],
                                            ident[:])
                        nc.vector.tensor_copy(VAs[bn][:, tb, 0:64],
                                              tp[:, 0:64])
                        nc.vector.tensor_copy(VAs[bn][:, tb, 65:129],
                                              tp[:, 64:128])

                def dup_half(hb):
                    hsl = ds(hb * SH, SH)
                    for t_i, (g, base) in enumerate(
                            ((0, 0), (1, 0), (0, 64), (1, 64))):
                        for blk in range(2):
                            for du in range(2):
                                nc.sync.dma_start(
                                    st["xd"][ds(64 * blk + 32 * du, 32),
                                             t_i, hsl],
                                    st["xs"][ds(base + 32 * blk, 32), g, hsl])

                def rope_q(cq):
                    qsl = ds(cq * QC, QC)
                    for xi, OUT in ((0, QRs[bn]), (2, KRs[bn])):
                        t1 = rpool.tile([128, QC], F32, tag=f"t1{xi}",
                                        name="t1")
                        t2 = rpool.tile([128, QC], F32, tag=f"t2{xi}",
                                        name="t2")
                        nc.gpsimd.tensor_mul(t1[:], st["xd"][:, xi, qsl],
                                             ra_t[:, qsl])
                        nc.vector.tensor_mul(t2[:], st["xd"][:, xi + 1, qsl],
                                             rb_t[:, qsl])
                        nc.vector.tensor_add(OUT[:, qsl], t1[:], t2[:])

                tasks = [alloc]
                for hb in range(2):
                    for ci in (2 * hb, 2 * hb + 1):
                        for half in range(2):
                            tasks.append(
                                lambda ci=ci, half=half: chain_v(ci, half))
                        for g in range(2):
                            for half in range(2):
                                tasks.append(lambda ci=ci, g=g, half=half:
                                             chain_x(ci, g, half))
                        tasks.append(lambda ci=ci: transpose_v(ci))
                    tasks.append(lambda hb=hb: dup_half(hb))
                    tasks.append(lambda hb=hb: rope_q(2 * hb))
                    tasks.append(lambda hb=hb: rope_q(2 * hb + 1))
                return tasks

            # ---- normalize half h of batch b ----------------------------
            def norm_recip(b, h):
                dbt = nrm.tile([2 * HPC, QC], F32, tag=f"dbt{h}",
                               name=f"dbt{h}")
                return dbt

            def norm_rest(b, h, dbt):
                rbf = nrm.tile([2 * HPC, QC], F32, tag=f"rbf{h}",
                               name=f"rbf{h}")
                rbt = nrm.tile([2 * HPC, QC], mm_dt,
                               tag=f"rbt{h}", name=f"rbt{h}")
                with nc.allow_low_precision(reason="softmax denom bcast"):
                    nc.vector.reciprocal_approx_fast(rbf[:], dbt[:])
                    nc.vector.tensor_copy(rbt[:], rbf[:])
                for qi in (2 * h, 2 * h + 1):
                    qsl = ds(qi * QC, QC)
                    bp = aux.tile([128, QC], F32, tag="aux", name="bp")
                    nc.tensor.matmul(bp[:], sels[:, qi, :], rbt[:],
                                     start=True, stop=True)
                    nc.vector.tensor_mul(ATTs[b][:, qsl], ATTs[b][:, qsl],
                                         bp[:])
                nc.sync.dma_start(attbs[b][h][:, :],
                                  ATTs[b][:, ds(h * SH, SH)])
                nc.gpsimd.collective_compute(
                    "AllGather", mybir.AluOpType.bypass,
                    replica_groups=[list(range(n_cores))],
                    ins=[attbs[b][h][:, :].opt()],
                    outs=[agbs[b][h][:, :].opt()])

            def norm_quarter3(qi, dbt):
                """Last batch, qi in (2,3): normalize + small quarter AG."""
                b = B - 1
                rbf = nrm.tile([HPC, QC], F32, tag="rbfq", name="rbfq")
                rbt = nrm.tile([HPC, QC], mm_dt, tag="rbtq", name="rbtq")
                with nc.allow_low_precision(reason="softmax denom bcast"):
                    nc.vector.reciprocal_approx_fast(rbf[:], dbt[:])
                    nc.vector.tensor_copy(rbt[:], rbf[:])
                qsl = ds(qi * QC, QC)
                bp = aux.tile([128, QC], F32, tag="aux", name="bp")
                nc.tensor.matmul(bp[:], selq[:, :], rbt[:],
                                 start=True, stop=True)
                nc.vector.tensor_mul(ATTs[b][:, qsl], ATTs[b][:, qsl], bp[:])
                nc.sync.dma_start(attbq[qi][:, :], ATTs[b][:, qsl])
                nc.gpsimd.collective_compute(
                    "AllGather", mybir.AluOpType.bypass,
                    replica_groups=[list(range(n_cores))],
                    ins=[attbq[qi][:, :].opt()], outs=[agbq[qi][:, :].opt()])

            # ---- one merged stream per batch ----------------------------
            # stream b = attention(b) + interleaved proj(b+1) + w_o(b-1)
            #            + deferred normalize
            DEPTH = 2
            proj0 = make_proj_tasks(0)
            proj0[0]()          # alloc + first xc DMA
            load_late_consts()  # rope tables etc. queue after xc(0)
            for t in proj0[1:]:
                t()

            carry_norm = None  # (b, h, dbt) deferred into next stream
            rt3 = {}
            for b in range(B):
                blocks = []
                for qi in range(nqc):
                    npair = (qi + 1) * NKP // 2
                    for hl in range(HPC):
                        for s in range(npair):
                            blocks.append((hl, qi, s, npair))
                ptasks = make_proj_tasks(b + 1) if b + 1 < B else []
                # interleave positions: proj task p after block idx
                # p*len(blocks)//len(ptasks)
                nxt_p = 0
                last = b == B - 1
                dbtA = norm_recip(b, 0)
                dbtB = norm_recip(b, 1)
                dbts = (dbtA, dbtB)
                if last:
                    dbtq = {_q: nrm.tile([HPC, QC], F32, tag=f"dbtq{_q}",
                                         name=f"dbtq{_q}") for _q in (2, 3)}
                pend = []  # (due_idx, fn)
                spq = [score_pair(b, *blocks[i][:3]) for i in range(DEPTH)]
                pv = None
                rt_pend = None
                for idx, (hl, qi, s, npair) in enumerate(blocks):
                    sp_cur = spq.pop(0)
                    if idx + DEPTH < len(blocks):
                        spq.append(score_pair(b, *blocks[idx + DEPTH][:3]))
                    pt = expmask_pair(b, hl, qi, s, sp_cur)
                    if s == 0:
                        pv = pvps.tile([65, QC], F32, tag="pv", name="pv")
                    pv_pair(b, hl, qi, s, pt, pv, npair * 2)
                    if s == npair - 1:  # finalize (qi, hl)
                        qsl = ds(qi * QC, QC)
                        nc.vector.tensor_copy(ATTs[b][ds(64 * hl, 64), qsl],
                                              pv[ds(0, 64), :])
                        deng = nrm.tile([1, QC], F32, tag="deng",
                                        name="deng")
                        nc.vector.tensor_copy(deng[:], pv[ds(64, 1), :])
                        if last and qi >= 2:
                            nc.sync.dma_start(dbtq[qi][ds(hl, 1), :],
                                              deng[:])
                        else:
                            nc.sync.dma_start(
                                dbts[qi // 2][ds((qi % 2) * HPC + hl, 1), :],
                                deng[:])
                        if last and qi >= 2 and hl == HPC - 1:
                            pend.append((idx + 1, lambda qi=qi:
                                         norm_quarter3(qi, dbtq[qi])))
                        if qi == 1 and hl == HPC - 1:
                            # half A fully done: normalize soon after
                            pend.append((idx + 2,
                                         lambda: norm_rest(b, 0, dbtA)))
                        # w_o of previous batch rides in this stream;
                        # for the last batch its own w_o starts in-stream
                        # as the quarter AllGathers land
                        if b > 0:
                            if hl == 0 and qi == 0:
                                rt_pend = wo_load(b - 1, 0)
                            elif hl == HPC - 1:
                                rt_next = (wo_load(b - 1, qi + 1)
                                           if qi + 1 < nqc else None)
                                wo_mm(b - 1, qi, rt_pend)
                                rt_pend = rt_next
                        # last batch: its own first-half w_o rides the
                        # end of its stream (AG h0 launched after qi=1)
                        if last and qi == 3:
                            if hl == 0:
                                rt3[0] = wo_load(b, 0)
                            else:
                                rt3[1] = wo_load(b, 1)
                                rt3[2] = wo_load(b, 2)
                                wo_mm(b, 0, rt3[0])
                    # deferred normalize from this or the previous stream
                    while pend and pend[0][0] <= idx:
                        pend.pop(0)[1]()
                    if carry_norm is not None and idx >= 1:
                        cb, ch, cdbt = carry_norm
                        carry_norm = None
                        norm_rest(cb, ch, cdbt)
                    # interleaved projection of the next batch
                    while (ptasks and
                           nxt_p < len(ptasks) and
                           idx >= nxt_p * (6 * len(blocks) // 10)
                           // len(ptasks)):
                        ptasks[nxt_p]()
                        nxt_p += 1
                while nxt_p < len(ptasks):
                    ptasks[nxt_p]()
                    nxt_p += 1
                for due, fn in pend:
                    fn()
                if b + 1 < B:
                    carry_norm = (b, 1, dbtB)

            # tail: only the last quarter's small AG is exposed
            wo_mm(B - 1, 1, rt3[1])
            rt3[3] = wo_load(B - 1, 3)
            wo_mm(B - 1, 2, rt3[2])
            wo_mm(B - 1, 3, rt3[3])

    nc.compile()
    return nc


def prep_inputs(inputs, seq=S, mm_dt=BF16, n_cores=NC):
    """Host-side sharding: build the per-core input maps."""
    mm_np = ml_dtypes.bfloat16 if mm_dt == BF16 else np.float32
    x = np.asarray(inputs["in_features"], dtype=np.float32)
    pos = np.asarray(inputs["token_positions"]).astype(np.float32)
    wq = np.asarray(inputs["w_q"], dtype=np.float32)
    wk = np.asarray(inputs["w_k"], dtype=np.float32)
    wv = np.asarray(inputs["w_v"], dtype=np.float32)
    wo = np.asarray(inputs["w_o"], dtype=np.float32)

    T = B * seq
    xT = np.ascontiguousarray(x.reshape(T, D).T).astype(mm_np)

    # rope tables (f32, matching reference numerics):
    #   QR = XA*A + XB*B ; A rows per 32-block: [cos, sin]*2 ;
    #   B rows: [-sin, cos]*2   (r2(t) = r1(t - pi/2))
    inv = np.float32(THETA) ** (-np.arange(0, DK, 2, dtype=np.float32)
                                / np.float32(DK))
    ang = pos[:, None].astype(np.float32) * inv[None, :].astype(np.float32)
    cosT = np.cos(ang.astype(np.float32)).T  # [32, seq]
    sinT = np.sin(ang.astype(np.float32)).T
    ropeA = np.ascontiguousarray(
        np.concatenate([cosT, sinT, cosT, sinT], axis=0)).astype(mm_np)
    ropeB = np.ascontiguousarray(
        np.concatenate([-sinT, cosT, -sinT, cosT], axis=0)).astype(mm_np)

    # normalize broadcast selectors (per seq half): out row m of chunk qi
    # takes denominator group g = (qi % 2) * HPC + (m // 64)
    nqc = seq // QC
    selT = np.zeros((2 * HPC, nqc, 128), dtype=mm_np)
    for qi in range(nqc):
        for m in range(128):
            selT[(qi % 2) * HPC + (m // 64), qi, m] = 1.0
    selQ = np.zeros((HPC, 128), dtype=mm_np)
    for m in range(128):
        selQ[m // 64, m] = 1.0

    ev = np.arange(0, DK, 2)
    od = ev + 1

    def lhsT_stack(W):
        # W [128 out-features, D] -> [128, NDB, 128]; [:, i, :] = W[:, 128i:+128].T
        Wt = np.ascontiguousarray(W.T).astype(mm_np)  # [D, 128]
        return np.ascontiguousarray(
            Wt.reshape(NDB, 128, 128).transpose(1, 0, 2))

    in_maps = []
    for c in range(n_cores):
        h0, h1 = HPC * c, HPC * c + 1
        W1 = np.concatenate([wq[DK * h0 + ev], wq[DK * h1 + ev],
                             wk[DK * h0 + ev], wk[DK * h1 + ev]], axis=0)
        W2 = np.concatenate([wq[DK * h0 + od], wq[DK * h1 + od],
                             wk[DK * h0 + od], wk[DK * h1 + od]], axis=0)
        wqkT = np.ascontiguousarray(np.stack(
            [lhsT_stack(Wg) for Wg in (W1, W2)], axis=1))
        WV = wv[128 * c: 128 * (c + 1)]
        WO = wo[128 * c: 128 * (c + 1)]
        in_maps.append({
            "xT": xT,
            "wqkT": wqkT,
            "wvT": lhsT_stack(WV),
            "woT": lhsT_stack(WO),
            "ropeA": ropeA,
            "ropeB": ropeB,
            "selT": selT,
            "selQ": selQ,
        })
    return in_maps


def assemble_output(results, seq=S, n_cores=NC):
    yT = np.concatenate([np.asarray(r["yT"], dtype=np.float32)
                         for r in results], axis=0)  # [1024, T]
    return np.ascontiguousarray(yT.T).reshape(B, seq, D).astype(np.float32)


_PROGRAM_CACHE = {}


def kernel(**inputs) -> np.ndarray:
    key = ("full", S, "bf16")
    if key not in _PROGRAM_CACHE:
        _PROGRAM_CACHE[key] = build_program(seq=S, mm_dt=BF16, n_cores=NC)
    nc = _PROGRAM_CACHE[key]
    in_maps = prep_inputs(inputs, seq=S, mm_dt=BF16, n_cores=NC)
    res = run_bass_kernel_spmd(nc, in_maps, list(range(NC)))
    return assemble_output(res.results)


# revision 37
# speedup vs baseline: 1.0471x; 1.0359x over previous
"""Multi-head self-attention (RoPE, causal) Trainium2 Bass kernel.

Problem: B=4, S=2048, D=1024, H=16 heads, d_k=64, f32 in/out.

Sharding: head-parallel across 8 NeuronCores. Core c owns heads {2c, 2c+1}
and all batches/tokens. QKV projections are column-parallel (each core
computes only its heads' features), attention is fully local per core, and
the output projection is column-parallel after per-(batch, seq-half)
AllGathers of the per-core attention outputs (each core computes 128 of
the 1024 output features).

Layouts (transposed activations, [feature, token]):
  - host pre-transposes x to xT [D, B*S]
  - Q/K projections produce de-interleaved features (x1 = even dims, x2 =
    odd dims, per head); batched SBUF->SBUF dup DMAs duplicate each 32-row
    group so each head's rope'd features are 64 contiguous rows, with
    QR = XA*A + XB*B against host-built phase-interleaved cos/sin tables;
    scores q.k are invariant under the shared q/k permutation
  - scores are computed transposed, S^T [k-partitions, q-free], so softmax
    exp output P^T feeds the P@V matmul with no transposes
  - V is projected transposed then PE-transposed to [token, feature] with a
    per-head ones column so each P@V matmul also emits the softmax
    denominator as output row 64
  - normalization per seq half: denominator rows gathered, one approximate
    reciprocal, selector-matmul broadcast to [128, QC], in-place multiply

Schedule (the perf-critical part; PE has a p-state ramp, so stalls cost
double — the whole program is one software-pipelined stream per batch):
  - stream b = attention(b) + interleaved projection chains of batch b+1
    (front-loaded into the first 70%) + w_o of batch b-1 + the deferred
    second-half normalize of batch b-1
  - attention runs over (q-chunk, head, k-block-pair) tiles with score
    matmuls issued two pairs ahead of the exp/mask/PV consumers, so the
    in-order PE queue never sits behind the scalar engine
  - exp is a single wide 1024-col Activation per untrimmed pair (3D AP
    flattened via rearrange); trimmed pairs share one trim offset
  - elementwise work is balanced: DVE does rope/masks/finalize copies,
    the scalar engine does exp + projection PSUM casts, Pool only fires
    collectives
  - AllGathers are per seq-half and launch mid-stream; w_o consumes them
    a full stream later, so collective latency and fabric traffic hide
    under compute
"""

import numpy as np
import ml_dtypes
from contextlib import ExitStack

import concourse.bass as bass
import concourse.bacc as bacc
import concourse.tile as tile
from concourse import mybir
from concourse.bass_utils import run_bass_kernel_spmd
from concourse.masks import make_upper_triangular, make_identity

F32 = mybir.dt.float32
BF16 = mybir.dt.bfloat16

B, S, D, H = 4, 2048, 1024, 16
NC = 8
DK = 64
HPC = H // NC  # heads per core
THETA = 100000.0
QC = 512       # q chunk (tokens per score-strip column block)
KB = 128       # k block (tokens per score-strip partition block)
NDB = D // 128  # number of 128-wide contraction blocks

ts = bass.ts
ds = bass.ds


def build_program(seq=S, mm_dt=BF16, n_cores=NC):
    """Build the SPMD Bass/Tile program (identical on every core)."""
    nc = bacc.Bacc("TRN2", target_bir_lowering=False, debug=False,
                   num_devices=n_cores)
    T = B * seq
    nqc = seq // QC
    nkb = seq // KB
    ntb = T // KB

    xT_h = nc.declare_dram_parameter("xT", [D, T], mm_dt, isOutput=False)
    wqk_h = nc.declare_dram_parameter("wqkT", [128, 2, NDB, 128], mm_dt,
                                      isOutput=False)
    wv_h = nc.declare_dram_parameter("wvT", [128, NDB, 128], mm_dt, isOutput=False)
    wo_h = nc.declare_dram_parameter("woT", [128, NDB, 128], mm_dt, isOutput=False)
    ra_h = nc.declare_dram_parameter("ropeA", [128, seq], mm_dt, isOutput=False)
    rb_h = nc.declare_dram_parameter("ropeB", [128, seq], mm_dt, isOutput=False)
    sel_h = nc.declare_dram_parameter("selT", [2 * HPC, nqc, 128],
                                      mm_dt, isOutput=False)
    selq_h = nc.declare_dram_parameter("selQ", [HPC, 128], mm_dt,
                                       isOutput=False)
    yT_h = nc.declare_dram_parameter("yT", [128, T], F32, isOutput=True)

    EXP = mybir.ActivationFunctionType.Exp
    scale = 1.0 / float(np.sqrt(DK))
    NKP = QC // KB  # k-blocks per q-chunk width

    with tile.TileContext(nc, num_cores=n_cores) as tc, ExitStack() as ctx:
        consts = ctx.enter_context(tc.tile_pool(name="consts", bufs=1))

        wqks = consts.tile([128, 2, NDB, 128], mm_dt)
        nc.sync.dma_start(wqks[:], wqk_h[:, :, :, :])
        wvs = consts.tile([128, NDB, 128], mm_dt)
        nc.sync.dma_start(wvs[:], wv_h[:, :, :])
        ra_t = consts.tile([128, seq], mm_dt)
        rb_t = consts.tile([128, seq], mm_dt)
        wos = consts.tile([128, NDB, 128], mm_dt)
        sels = consts.tile([2 * HPC, nqc, 128], mm_dt)
        selq = consts.tile([HPC, 128], mm_dt)

        def load_late_consts():
            nc.sync.dma_start(ra_t[:], ra_h[:, :])
            nc.sync.dma_start(rb_t[:], rb_h[:, :])
            nc.sync.dma_start(wos[:], wo_h[:, :, :])
            nc.sync.dma_start(sels[:], sel_h[:, :, :])
            nc.sync.dma_start(selq[:], selq_h[:, :])

        triu = consts.tile([128, 128], mm_dt)
        make_upper_triangular(nc, triu[:], val=1.0, diag=True)
        ident = consts.tile([128, 128], mm_dt)
        make_identity(nc, ident[:])

        NTB = seq // KB   # 128-token V blocks per batch
        SH = seq // 2     # half-batch tokens

        xpool = ctx.enter_context(tc.tile_pool(name="xcp", bufs=2))
        stg = ctx.enter_context(tc.tile_pool(name="stage", bufs=1))
        dpup = ctx.enter_context(tc.tile_pool(name="dupt", bufs=1))
        rpool = ctx.enter_context(tc.tile_pool(name="ropet", bufs=4))
        vtp = ctx.enter_context(tc.tile_pool(name="vtcp", bufs=2))
        qrp = ctx.enter_context(tc.tile_pool(name="qrp", bufs=2))
        krp = ctx.enter_context(tc.tile_pool(name="krp", bufs=2))
        vap = ctx.enter_context(tc.tile_pool(name="vap", bufs=2))
        attp = ctx.enter_context(tc.tile_pool(name="attp", bufs=2))
        sps = ctx.enter_context(tc.tile_pool(name="sps", bufs=2, space="PSUM"))
        pvps = ctx.enter_context(tc.tile_pool(name="pvps", bufs=2,
                                              space="PSUM"))
        aux = ctx.enter_context(tc.tile_pool(name="auxps", bufs=2,
                                             space="PSUM"))
        ptp = ctx.enter_context(tc.tile_pool(name="ptp", bufs=4))
        nrm = ctx.enter_context(tc.tile_pool(name="nrm", bufs=2))
        dpool = ctx.enter_context(tc.tile_pool(name="dram", bufs=1,
                                               space="DRAM"))
        agp = ctx.enter_context(tc.tile_pool(name="agp", bufs=3))
        ysb = ctx.enter_context(tc.tile_pool(name="ysb", bufs=2))
        if True:
            # per-(batch, half) collective buffers
            attbs = [[dpool.tile([128, SH], mm_dt, name=f"attb{_b}h{_h}")
                      for _h in range(2)] for _b in range(B)]
            agbs = [[dpool.tile([128 * n_cores, SH], mm_dt,
                                addr_space="Shared", name=f"agb{_b}h{_h}")
                     for _h in range(2)] for _b in range(B)]

            xr = xT_h[:, :].rearrange("(i p) t -> p i t", p=128)

            # per-batch rotating tiles (dicts keyed by batch)
            QRs, KRs, VAs, ATTs = {}, {}, {}, {}

            # ---- w_o helpers (chunk cc of batch bb) ----------------------
            def wo_load(bb, cc):
                agr = agbs[bb][cc // 2][:, :].rearrange(
                    "(i p) t -> p i t", p=128)
                rt = agp.tile([128, NDB, QC], mm_dt, tag="rt", name="rt")
                nc.sync.dma_start(rt[:], agr[:, :, ts(cc % 2, QC)])
                return rt

            def wo_mm(bb, cc, rt):
                yp = aux.tile([128, QC], F32, tag="aux", name="yp")
                for i in range(NDB):
                    nc.tensor.matmul(yp[:], wos[:, i, :], rt[:, i, :],
                                     start=(i == 0), stop=(i == NDB - 1))
                ysb_t = ysb.tile([128, QC], F32, tag="ys", name="ysb_t")
                nc.vector.tensor_copy(ysb_t[:], yp[:])
                nc.sync.dma_start(yT_h[:, ds(bb * seq + cc * QC, QC)],
                                  ysb_t[:])

            # ---- attention pair helpers (2 k-blocks per PSUM tile) ------
            def score_pair(b, hl, qi, s):
                hr = ds(64 * hl, 64)
                sp = sps.tile([128, 2, QC], F32, tag="sp", name="sp")
                for j in range(2):
                    kb = 2 * s + j
                    ksl = ds(kb * KB, KB)
                    dj = kb - qi * NKP
                    qv = 128 * dj if dj > 0 else 0
                    qslv = ds(qi * QC + qv, QC - qv)
                    nc.tensor.matmul(sp[:, j, qv:QC], KRs[b][hr, ksl],
                                     QRs[b][hr, qslv], start=True, stop=True)
                return sp

            def expmask_pair(b, hl, qi, s, sp):
                pt = ptp.tile([128, 2, QC], mm_dt, tag="pt", name="pt")
                djs = [2 * s + j - qi * NKP for j in range(2)]
                qvm = max(0, min(djs)) * 128
                if qvm == 0:  # one wide 2D activation over the whole pair
                    nc.scalar.activation(
                        pt[:, :, :].rearrange("p a c -> p (a c)"),
                        sp[:, :, :].rearrange("p a c -> p (a c)"),
                        EXP, scale=scale)
                else:
                    nc.scalar.activation(pt[:, :, qvm:QC], sp[:, :, qvm:QC],
                                         EXP, scale=scale)
                for j in range(2):
                    if djs[j] >= 0:  # diagonal block: causal mask
                        dsl = ds(128 * djs[j], 128)
                        nc.vector.tensor_mul(pt[:, j, dsl], pt[:, j, dsl],
                                             triu[:])
                return pt

            def pv_pair(b, hl, qi, s, pt, pv, nk):
                for j in range(2):
                    kb = 2 * s + j
                    dj = kb - qi * NKP
                    qv = 128 * dj if dj > 0 else 0
                    nc.tensor.matmul(pv[:, qv:QC],
                                     VAs[b][:, kb, ds(65 * hl, 65)],
                                     pt[:, j, qv:QC],
                                     start=(kb == 0), stop=(kb == nk - 1))

            # ---- projection task list for batch bn ----------------------
            def make_proj_tasks(bn):
                """Return a list of callables; run in order (interleaved)."""
                st = {"xcs": {}, "xs": None, "xd": None}

                def alloc():
                    QRs[bn] = qrp.tile([128, seq], mm_dt, tag="qr",
                                       name=f"qr{bn}")
                    KRs[bn] = krp.tile([128, seq], mm_dt, tag="kr",
                                       name=f"kr{bn}")
                    VAs[bn] = vap.tile([128, NTB, 130], mm_dt, tag="va",
                                       name=f"va{bn}")
                    ATTs[bn] = attp.tile([128, seq], mm_dt, tag="att",
                                         name=f"att{bn}")
                    nc.vector.memset(VAs[bn][:, :, 64], 1.0)
                    nc.vector.memset(VAs[bn][:, :, 129], 1.0)
                    st["xs"] = stg.tile([128, 2, seq], mm_dt, tag="xs",
                                        name="xs")
                    st["xd"] = dpup.tile([128, 4, seq], mm_dt, tag="xd",
                                         name="xd")
                    xc = xpool.tile([128, NDB, QC], mm_dt, tag="xc",
                                    name="xc")
                    nc.sync.dma_start(xc[:], xr[:, :, ts(bn * nqc, QC)])
                    st["xcs"][0] = xc

                def chain_v(ci, half):
                    if half == 0:
                        if ci + 1 < nqc:  # prefetch next chunk
                            xc = xpool.tile([128, NDB, QC], mm_dt, tag="xc",
                                            name="xc")
                            nc.sync.dma_start(
                                xc[:], xr[:, :, ts(bn * nqc + ci + 1, QC)])
                            st["xcs"][ci + 1] = xc
                        st["psv"] = aux.tile([128, QC], F32, tag="aux",
                                             name="psv")
                    psv = st["psv"]
                    for i in range(4 * half, 4 * half + 4):
                        nc.tensor.matmul(psv[:], wvs[:, i, :],
                                         st["xcs"][ci][:, i, :],
                                         start=(i == 0), stop=(i == NDB - 1))
                    if half == 1:
                        vtc = vtp.tile([128, QC], mm_dt, tag="vtc",
                                       name="vtc")
                        nc.scalar.copy(vtc[:], psv[:])
                        st[f"vtc{ci}"] = vtc

                def chain_x(ci, g, half):
                    if half == 0:
                        st["ps"] = aux.tile([128, QC], F32, tag="aux",
                                            name="ps")
                    ps = st["ps"]
                    for i in range(4 * half, 4 * half + 4):
                        nc.tensor.matmul(ps[:], wqks[:, g, i, :],
                                         st["xcs"][ci][:, i, :],
                                         start=(i == 0), stop=(i == NDB - 1))
                    if half == 1:
                        nc.vector.tensor_copy(st["xs"][:, g, ds(ci * QC, QC)],
                                              ps[:])

                def transpose_v(ci):
                    vtc = st[f"vtc{ci}"]
                    for sb in range(QC // 128):
                        tb = ci * (QC // 128) + sb
                        tp = aux.tile([128, 128], mm_dt, tag="aux", name="tp")
                        nc.tensor.transpose(tp[:], vtc[:, ts(sb, 128)],
                                            ident[:])
                        nc.vector.tensor_copy(VAs[bn][:, tb, 0:64],
                                              tp[:, 0:64])
                        nc.vector.tensor_copy(VAs[bn][:, tb, 65:129],
                                              tp[:, 64:128])

                def dup_half(hb):
                    hsl = ds(hb * SH, SH)
                    for t_i, (g, base) in enumerate(
                            ((0, 0), (1, 0), (0, 64), (1, 64))):
                        for blk in range(2):
                            for du in range(2):
                                nc.sync.dma_start(
                                    st["xd"][ds(64 * blk + 32 * du, 32),
                                             t_i, hsl],
                                    st["xs"][ds(base + 32 * blk, 32), g, hsl])

                def rope_q(cq):
                    qsl = ds(cq * QC, QC)
                    for xi, OUT in ((0, QRs[bn]), (2, KRs[bn])):
                        t1 = rpool.tile([128, QC], F32, tag=f"t1{xi}",
                                        name="t1")
                        t2 = rpool.tile([128, QC], F32, tag=f"t2{xi}",
                                        name="t2")
                        nc.gpsimd.tensor_mul(t1[:], st["xd"][:, xi, qsl],
                                             ra_t[:, qsl])
                        nc.vector.tensor_mul(t2[:], st["xd"][:, xi + 1, qsl],
                                             rb_t[:, qsl])
                        nc.vector.tensor_add(OUT[:, qsl], t1[:], t2[:])

                tasks = [alloc]
                for hb in range(2):
                    for ci in (2 * hb, 2 * hb + 1):
                        for half in range(2):
                            tasks.append(
                                lambda ci=ci, half=half: chain_v(ci, half))
                        for g in range(2):
                            for half in range(2):
                                tasks.append(lambda ci=ci, g=g, half=half:
                                             chain_x(ci, g, half))
                        tasks.append(lambda ci=ci: transpose_v(ci))
                    tasks.append(lambda hb=hb: dup_half(hb))
                    tasks.append(lambda hb=hb: rope_q(2 * hb))
                    tasks.append(lambda hb=hb: rope_q(2 * hb + 1))
                return tasks

            # ---- normalize half h of batch b ----------------------------
            def norm_recip(b, h):
                dbt = nrm.tile([2 * HPC, QC], F32, tag=f"dbt{h}",
                               name=f"dbt{h}")
                return dbt

            def norm_rest(b, h, dbt):
                rbf = nrm.tile([2 * HPC, QC], F32, tag=f"rbf{h}",
                               name=f"rbf{h}")
                rbt = nrm.tile([2 * HPC, QC], mm_dt,
                               tag=f"rbt{h}", name=f"rbt{h}")
                with nc.allow_low_precision(reason="softmax denom bcast"):
                    nc.vector.reciprocal_approx_fast(rbf[:], dbt[:])
                    nc.vector.tensor_copy(rbt[:], rbf[:])
                for qi in (2 * h, 2 * h + 1):
                    qsl = ds(qi * QC, QC)
                    bp = aux.tile([128, QC], F32, tag="aux", name="bp")
                    nc.tensor.matmul(bp[:], sels[:, qi, :], rbt[:],
                                     start=True, stop=True)
                    nc.vector.tensor_mul(ATTs[b][:, qsl], ATTs[b][:, qsl],
                                         bp[:])
                nc.sync.dma_start(attbs[b][h][:, :],
                                  ATTs[b][:, ds(h * SH, SH)])
                nc.gpsimd.collective_compute(
                    "AllGather", mybir.AluOpType.bypass,
                    replica_groups=[list(range(n_cores))],
                    ins=[attbs[b][h][:, :].opt()],
                    outs=[agbs[b][h][:, :].opt()])

            # ---- one merged stream per batch ----------------------------
            # stream b = attention(b) + interleaved proj(b+1) + w_o(b-1)
            #            + deferred normalize
            DEPTH = 2
            proj0 = make_proj_tasks(0)
            proj0[0]()          # alloc + first xc DMA
            load_late_consts()  # rope tables etc. queue after xc(0)
            for t in proj0[1:]:
                t()

            carry_norm = None  # (b, h, dbt) deferred into next stream
            rt3 = {}
            for b in range(B):
                blocks = []
                for qi in range(nqc):
                    npair = (qi + 1) * NKP // 2
                    for hl in range(HPC):
                        for s in range(npair):
                            blocks.append((hl, qi, s, npair))
                ptasks = make_proj_tasks(b + 1) if b + 1 < B else []
                # interleave positions: proj task p after block idx
                # p*len(blocks)//len(ptasks)
                nxt_p = 0
                last = b == B - 1
                dbtA = norm_recip(b, 0)
                dbtB = norm_recip(b, 1)
                dbts = (dbtA, dbtB)
                pend = []  # (due_idx, fn)
                spq = [score_pair(b, *blocks[i][:3]) for i in range(DEPTH)]
                pv = None
                rt_pend = None
                for idx, (hl, qi, s, npair) in enumerate(blocks):
                    sp_cur = spq.pop(0)
                    if idx + DEPTH < len(blocks):
                        spq.append(score_pair(b, *blocks[idx + DEPTH][:3]))
                    pt = expmask_pair(b, hl, qi, s, sp_cur)
                    if s == 0:
                        pv = pvps.tile([65, QC], F32, tag="pv", name="pv")
                    pv_pair(b, hl, qi, s, pt, pv, npair * 2)
                    if s == npair - 1:  # finalize (qi, hl)
                        qsl = ds(qi * QC, QC)
                        nc.vector.tensor_copy(ATTs[b][ds(64 * hl, 64), qsl],
                                              pv[ds(0, 64), :])
                        deng = nrm.tile([1, QC], F32, tag="deng",
                                        name="deng")
                        nc.vector.tensor_copy(deng[:], pv[ds(64, 1), :])
                        nc.sync.dma_start(
                            dbts[qi // 2][ds((qi % 2) * HPC + hl, 1), :],
                            deng[:])
                        if qi == 1 and hl == HPC - 1:
                            # half A fully done: normalize soon after
                            pend.append((idx + 2,
                                         lambda: norm_rest(b, 0, dbtA)))
                        # w_o of previous batch rides in this stream;
                        # for the last batch its own w_o starts in-stream
                        # as the quarter AllGathers land
                        if b > 0:
                            if hl == 0 and qi == 0:
                                rt_pend = wo_load(b - 1, 0)
                            elif hl == HPC - 1:
                                rt_next = (wo_load(b - 1, qi + 1)
                                           if qi + 1 < nqc else None)
                                wo_mm(b - 1, qi, rt_pend)
                                rt_pend = rt_next
                        # last batch: its own first-half w_o rides the
                        # end of its stream (AG h0 launched after qi=1)
                        if last and qi == 3:
                            if hl == 0:
                                rt3[0] = wo_load(b, 0)
                            else:
                                rt3[1] = wo_load(b, 1)
                                wo_mm(b, 0, rt3[0])
                    # deferred normalize from this or the previous stream
                    while pend and pend[0][0] <= idx:
                        pend.pop(0)[1]()
                    if carry_norm is not None and idx >= 1:
                        cb, ch, cdbt = carry_norm
                        carry_norm = None
                        norm_rest(cb, ch, cdbt)
                    # interleaved projection of the next batch
                    while (ptasks and
                           nxt_p < len(ptasks) and
                           idx >= nxt_p * (6 * len(blocks) // 10)
                           // len(ptasks)):
                        ptasks[nxt_p]()
                        nxt_p += 1
                while nxt_p < len(ptasks):
                    ptasks[nxt_p]()
                    nxt_p += 1
                for due, fn in pend:
                    fn()
                if b + 1 < B:
                    carry_norm = (b, 1, dbtB)
                else:
                    norm_rest(b, 1, dbtB)

            # tail: second-half w_o of the last batch (loads ahead)
            wo_mm(B - 1, 1, rt3[1])
            rt3[2] = wo_load(B - 1, 2)
            rt3[3] = wo_load(B - 1, 3)
            wo_mm(B - 1, 2, rt3[2])
            wo_mm(B - 1, 3, rt3[3])

    nc.compile()
    return nc


def prep_inputs(inputs, seq=S, mm_dt=BF16, n_cores=NC):
    """Host-side sharding: build the per-core input maps."""
    mm_np = ml_dtypes.bfloat16 if mm_dt == BF16 else np.float32
    x = np.asarray(inputs["in_features"], dtype=np.float32)
    pos = np.asarray(inputs["token_positions"]).astype(np.float32)
    wq = np.asarray(inputs["w_q"], dtype=np.float32)
    wk = np.asarray(inputs["w_k"], dtype=np.float32)
    wv = np.asarray(inputs["w_v"], dtype=np.float32)
    wo = np.asarray(inputs["w_o"], dtype=np.float32)

    T = B * seq
    xT = np.ascontiguousarray(x.reshape(T, D).T).astype(mm_np)

    # rope tables (f32, matching reference numerics):
    #   QR = XA*A + XB*B ; A rows per 32-block: [cos, sin]*2 ;
    #   B rows: [-sin, cos]*2   (r2(t) = r1(t - pi/2))
    inv = np.float32(THETA) ** (-np.arange(0, DK, 2, dtype=np.float32)
                                / np.float32(DK))
    ang = pos[:, None].astype(np.float32) * inv[None, :].astype(np.float32)
    cosT = np.cos(ang.astype(np.float32)).T  # [32, seq]
    sinT = np.sin(ang.astype(np.float32)).T
    ropeA = np.ascontiguousarray(
        np.concatenate([cosT, sinT, cosT, sinT], axis=0)).astype(mm_np)
    ropeB = np.ascontiguousarray(
        np.concatenate([-sinT, cosT, -sinT, cosT], axis=0)).astype(mm_np)

    # normalize broadcast selectors (per seq half): out row m of chunk qi
    # takes denominator group g = (qi % 2) * HPC + (m // 64)
    nqc = seq // QC
    selT = np.zeros((2 * HPC, nqc, 128), dtype=mm_np)
    for qi in range(nqc):
        for m in range(128):
            selT[(qi % 2) * HPC + (m // 64), qi, m] = 1.0
    selQ = np.zeros((HPC, 128), dtype=mm_np)
    for m in range(128):
        selQ[m // 64, m] = 1.0

    ev = np.arange(0, DK, 2)
    od = ev + 1

    def lhsT_stack(W):
        # W [128 out-features, D] -> [128, NDB, 128]; [:, i, :] = W[:, 128i:+128].T
        Wt = np.ascontiguousarray(W.T).astype(mm_np)  # [D, 128]
        return np.ascontiguousarray(
            Wt.reshape(NDB, 128, 128).transpose(1, 0, 2))

    in_maps = []
    for c in range(n_cores):
        h0, h1 = HPC * c, HPC * c + 1
        W1 = np.concatenate([wq[DK * h0 + ev], wq[DK * h1 + ev],
                             wk[DK * h0 + ev], wk[DK * h1 + ev]], axis=0)
        W2 = np.concatenate([wq[DK * h0 + od], wq[DK * h1 + od],
                             wk[DK * h0 + od], wk[DK * h1 + od]], axis=0)
        wqkT = np.ascontiguousarray(np.stack(
            [lhsT_stack(Wg) for Wg in (W1, W2)], axis=1))
        WV = wv[128 * c: 128 * (c + 1)]
        WO = wo[128 * c: 128 * (c + 1)]
        in_maps.append({
            "xT": xT,
            "wqkT": wqkT,
            "wvT": lhsT_stack(WV),
            "woT": lhsT_stack(WO),
            "ropeA": ropeA,
            "ropeB": ropeB,
            "selT": selT,
            "selQ": selQ,
        })
    return in_maps


def assemble_output(results, seq=S, n_cores=NC):
    yT = np.concatenate([np.asarray(r["yT"], dtype=np.float32)
                         for r in results], axis=0)  # [1024, T]
    return np.ascontiguousarray(yT.T).reshape(B, seq, D).astype(np.float32)


_PROGRAM_CACHE = {}


def kernel(**inputs) -> np.ndarray:
    key = ("full", S, "bf16")
    if key not in _PROGRAM_CACHE:
        _PROGRAM_CACHE[key] = build_program(seq=S, mm_dt=BF16, n_cores=NC)
    nc = _PROGRAM_CACHE[key]
    in_maps = prep_inputs(inputs, seq=S, mm_dt=BF16, n_cores=NC)
    res = run_bass_kernel_spmd(nc, in_maps, list(range(NC)))
    return assemble_output(res.results)
